# revision 31
# baseline (speedup 1.0000x reference)
"""MeanNSE (segment-reduce) Trainium2 kernel — 8 NeuronCores, data-parallel.

Strategy: the basin ids are pure index data, so all index math runs on the
host; the device does every FLOP over the 16.7M-element float arrays.

Host: stable-sort elements by basin and zero-pad each basin's run to a
multiple of W so that every W-element "row" of the padded layout belongs to
exactly one basin (pad elements are zeros in both y_true and y_pred and
therefore contribute exactly 0 to every partial sum). The padded layout is
split evenly across the 8 cores.

Device (per core): stream rows through SBUF in bf16 and emit three f32
partial sums per row — sum(t), sum(t^2), sum((t-p)^2) — using only dense
contiguous-reduction instructions, load-balanced across three engines
(measured rates: GPSIMD sub ~1.9ns/el, DVE reduce-ops ~1.2ns/el, ACT
~1.4us/1024-block; DMA ~28us for the 8.9MB of bf16 inputs):
  - GPSIMD: d = t - p (elementwise, otherwise idle engine)
  - DVE: scalar_tensor_tensor(d*d, accum_out) for all sum(d^2) rows and
    tensor_reduce(axis=X) for a slice of the sum(t) rows
  - Scalar/Act: activation(Square, accum_out) for t^2; activation(Copy,
    accum_out) for the rest of the t row-sums
(tensor_tensor_reduce is avoided: it hard-crashes the NEFF on this
runtime; scalar_tensor_tensor's accumulator path is the working variant.)
Inputs stream over both hardware DGE queues (sync + scalar engines). All
tiles fit in SBUF, so every input DMA is issued up front.

Host: map the tiny [rows] sums back to basins (bincount), combine in
float64 with exact integer counts: ss_tot = sum_t2 - sum_t^2/count,
nse = 1 - ss_res/(ss_tot + 1e-10), answer = mean over 671 basins.

bf16 input rounding perturbs each value by ~2^-9 relative; the final
mean-NSE error stays ~1e-4, far inside the 2e-2 gate.
"""

import sys

sys.path.insert(0, "/opt/trn_rl_repo")

import numpy as np
import ml_dtypes

import concourse.bacc as bacc
import concourse.mybir as mybir
import concourse.tile as tile
from concourse.bass_utils import run_bass_kernel_spmd

F32 = mybir.dt.float32
BF16 = mybir.dt.bfloat16
BF16_NP = ml_dtypes.bfloat16

N_CORES = 8
N_TOTAL = 16777216
N_BASINS = 671
EPS = 1e-10

W = 1024  # row width (elements); every row belongs to one basin
# Worst-case rows: ceil((N + 671*(W-1)) / W), rounded so each core gets an
# identical whole number of 128-row groups.
_R_MAX = -(-(N_TOTAL + N_BASINS * (W - 1)) // W)
R_C = -(-_R_MAX // (N_CORES * 128)) * 128  # rows per core (2176)
J = R_C // 128  # 128-row j-blocks per core (17)
E_C = R_C * W  # elements per core (2,228,224)
# size-graded tiles: small first tiles start the compute pipeline early
K_PLAN = [1, 1, 2, 2, 2, 3, 3, 3]  # sums to J=17

# j-blocks whose sum(t) is produced on the DVE (the rest on the scalar
# engine) — balances the two engines' instruction streams.
T_ON_DVE = frozenset({2, 6, 10, 12, 15})
# DMA queue assignment: (array, tile) -> queue. All yt on the sync-engine
# HW queue (it paces the scalar engine's activations), early/late yp on
# the scalar-engine HW queue (dispatched before any activation), middle yp
# on the gpsimd software-DGE queue. Measured best of several splits.
SYNC_Q = tuple(("yt", t) for t in range(len(K_PLAN)))
SCALAR_Q = (("yp", 0), ("yp", 1), ("yp", 6), ("yp", 4))
GPSIMD_Q = (("yp", 2), ("yp", 3), ("yp", 5), ("yp", 7))

_AF = mybir.ActivationFunctionType
_ALU = mybir.AluOpType

_cache = {}


def _build():
    nc = bacc.Bacc()
    yt = nc.declare_dram_parameter("yt", [E_C], BF16, isOutput=False)
    yp = nc.declare_dram_parameter("yp", [E_C], BF16, isOutput=False)
    # out planes: 0 = sum_t (DVE cols), 1 = sum_t (scalar cols),
    #             2 = sum_t2, 3 = sum_d2
    out = nc.declare_dram_parameter("out", [4 * 128 * J], F32, isOutput=True)

    with tile.TileContext(nc) as tc:
        with (
            tc.tile_pool(name="cpool", bufs=1) as cpool,
            tc.tile_pool(name="io", bufs=1) as io_pool,
            tc.tile_pool(name="dpool", bufs=2) as d_pool,
        ):
            # one output tile; plane s occupies columns [s*J, (s+1)*J):
            # 0 = sum_t (DVE cols), 1 = sum_t (ACT cols), 2 = sum_t2, 3 = sum_d2
            sums = cpool.tile([128, 4 * J], F32, tag="sums")
            scr_v = cpool.tile([128, W], BF16, tag="scr_v")
            scr_a = cpool.tile([128, W], BF16, tag="scr_a")
            # unwritten columns of the split sum_t planes must not be NaN
            nc.vector.memset(sums[:, 0:J], 0.0)
            nc.scalar.memzero(sums[:, J : 2 * J])

            # stage all input tiles up front (everything fits in SBUF),
            # spread over three DMA queues: yt on the sync-engine HW queue
            # (paces the scalar engine's work), three yp tiles on the
            # scalar-engine HW queue (dispatched before any activation),
            # the middle yp tiles on the gpsimd software-DGE queue
            tiles = []
            slices = []
            base = 0
            for t, k in enumerate(K_PLAN):
                n_el = 128 * k * W
                tt_ = io_pool.tile([128, k * W], BF16, tag=f"yt{t}")
                tp_ = io_pool.tile([128, k * W], BF16, tag=f"yp{t}")
                sl = lambda x, b=base, n=n_el, kk=k: x[b : b + n].rearrange(
                    "(p f) -> p f", p=128, f=kk * W
                )
                tiles.append((tt_, tp_, k))
                slices.append(sl)
                base += n_el

            def _dst(arr, t):
                return tiles[t][0 if arr == "yt" else 1][:, :]

            def _src(arr, t):
                return slices[t](yt if arr == "yt" else yp)

            for arr, t in SYNC_Q:
                nc.sync.dma_start(_dst(arr, t), _src(arr, t))
            for arr, t in SCALAR_Q:
                nc.scalar.dma_start(_dst(arr, t), _src(arr, t))
            for arr, t in GPSIMD_Q:
                nc.gpsimd.dma_start(_dst(arr, t), _src(arr, t))

            jj = 0
            for tt_, tp_, k in tiles:
                # DVE t-sums first: they need only yt, so they fill the
                # vector engine while yp is still in flight
                for j in range(k):
                    if jj + j in T_ON_DVE:
                        nc.vector.tensor_reduce(
                            sums[:, jj + j : jj + j + 1],
                            tt_[:, j * W : (j + 1) * W],
                            axis=mybir.AxisListType.X,
                            op=_ALU.add,
                        )
                d_t = d_pool.tile([128, k * W], BF16, tag="d")
                nc.vector.tensor_sub(d_t[:, :], tt_[:, :], tp_[:, :])
                for j in range(k):
                    sl = slice(j * W, (j + 1) * W)
                    nc.vector.scalar_tensor_tensor(
                        out=scr_v[:, :],
                        in0=d_t[:, sl],
                        scalar=0.0,
                        in1=d_t[:, sl],
                        op0=_ALU.add,
                        op1=_ALU.mult,
                        accum_out=sums[:, 3 * J + jj : 3 * J + jj + 1],
                    )
                    nc.scalar.activation(
                        scr_a[:, :],
                        tt_[:, sl],
                        _AF.Square,
                        accum_out=sums[:, 2 * J + jj : 2 * J + jj + 1],
                    )
                    if jj not in T_ON_DVE:
                        nc.scalar.activation(
                            scr_a[:, :],
                            tt_[:, sl],
                            _AF.Copy,
                            accum_out=sums[:, J + jj : J + jj + 1],
                        )
                    jj += 1

            nc.sync.dma_start(
                out[:].rearrange("(p x) -> p x", p=128, x=4 * J),
                sums[:, :],
            )
    nc.compile()
    return nc


def _get_nc():
    if "nc" not in _cache:
        _cache["nc"] = _build()
    return _cache["nc"]


def _row_map():
    """local row index for (partition p, j-block jj) within one core."""
    m = np.empty((128, J), np.int64)
    jb = 0
    base = 0
    for k in K_PLAN:
        m[:, jb : jb + k] = (
            base + np.arange(128)[:, None] * k + np.arange(k)[None, :]
        )
        jb += k
        base += 128 * k
    return m


def _prepare(y_pred, y_true, basin):
    """Host-side index math: sort by basin, zero-pad to W-multiples."""
    y_pred = np.asarray(y_pred, dtype=np.float32)
    y_true = np.asarray(y_true, dtype=np.float32)
    b = np.asarray(basin).astype(np.int32)
    n = b.shape[0]

    counts = np.bincount(b, minlength=N_BASINS)
    pc = (counts + W - 1) // W * W  # per-basin padded counts
    pad_off = np.zeros(N_BASINS + 1, np.int64)
    np.cumsum(pc, out=pad_off[1:])
    P = int(pad_off[-1])
    assert P <= N_CORES * E_C, (P, N_CORES * E_C)

    order = np.argsort(b, kind="stable")
    seg_start = np.zeros(N_BASINS, np.int64)
    np.cumsum(counts[:-1], out=seg_start[1:])
    bs = b[order]
    dst = pad_off[bs] + (np.arange(n, dtype=np.int64) - seg_start[bs])

    yt_pad = np.zeros(N_CORES * E_C, dtype=BF16_NP)
    yp_pad = np.zeros(N_CORES * E_C, dtype=BF16_NP)
    yt_pad[dst] = y_true[order].astype(BF16_NP)
    yp_pad[dst] = y_pred[order].astype(BF16_NP)
    yt_pad = yt_pad.reshape(N_CORES, E_C)
    yp_pad = yp_pad.reshape(N_CORES, E_C)

    in_maps = [{"yt": yt_pad[c], "yp": yp_pad[c]} for c in range(N_CORES)]

    # basin of every global row (pad rows -> N_BASINS, dropped later)
    row_basin = np.full(N_CORES * R_C, N_BASINS, np.int64)
    rb = np.repeat(np.arange(N_BASINS), pc // W)
    row_basin[: rb.shape[0]] = rb
    return in_maps, (counts, row_basin)


def _finish(results, ctx):
    counts, row_basin = ctx
    rmap = _row_map()
    rows = np.empty((3, N_CORES * R_C), np.float64)
    for c in range(N_CORES):
        arr = np.asarray(results[c]["out"], np.float64).reshape(128, 4, J)
        arr = arr.transpose(1, 0, 2)  # -> [plane, p, jj]
        sl = slice(c * R_C, (c + 1) * R_C)
        for s, plane in enumerate((arr[0] + arr[1], arr[2], arr[3])):
            dest = np.empty(R_C, np.float64)
            dest[rmap.ravel()] = plane.ravel()
            rows[s, sl] = dest
    s_t = np.bincount(row_basin, weights=rows[0], minlength=N_BASINS + 1)[
        :N_BASINS
    ]
    s_t2 = np.bincount(row_basin, weights=rows[1], minlength=N_BASINS + 1)[
        :N_BASINS
    ]
    s_d2 = np.bincount(row_basin, weights=rows[2], minlength=N_BASINS + 1)[
        :N_BASINS
    ]
    cnt = counts.astype(np.float64)
    ss_tot = s_t2 - s_t * s_t / cnt
    nse = 1.0 - s_d2 / (ss_tot + EPS)
    return np.float32(nse.mean())


def kernel(y_pred, y_true, basin):
    in_maps, ctx = _prepare(y_pred, y_true, basin)
    res = run_bass_kernel_spmd(_get_nc(), in_maps, list(range(N_CORES)))
    return _finish(res.results, ctx)
